# revision 9
# baseline (speedup 1.0000x reference)
"""Trainium2 Bass kernel for nn_Attention (B,H,S,D = 4,16,2048,64):
scores = (QK^T + QT^T)/sqrt(D) (attn_mask all-ones), p_attn =
softmax(scores) * sigmoid(mask_logits), out = p_attn @ V.
Returns (out, p_attn) like the reference.

Sharding: B*H = 64 heads split across 8 NeuronCores (8 heads/core).
Per core, loop supertiles of 512 queries (js) outer, head (h) inner.
Engines (raw bass, manual semaphores — this toolchain allows only ONE
semaphore wait per compute instruction, so waits are standalone wait_ge
instructions and the whole pipeline is hand-scheduled):
 - PE: natural-layout scores matmuls in fp32r (e8m11 -> 1 cyc/row at
   N=512, products exact in fp32 accumulate), PE transposes of the
   normalized p tiles, AV matmuls accumulating out^T [64,512] over
   k-chunks. Scores of iteration i are interleaved with the
   transpose/AV phase of iteration i-1 (1-deep software pipeline).
 - ACT: exp(scores/8) straight from PSUM with free rowsum accum_out,
   plus most of the transposed-p PSUM->SBUF copies.
 - DVE: reciprocal + fused (u * inv) * sigmoid in one pass writing p in
   fp32r, the remaining copies, and the out^T PSUM->SBUF copy.
 - SP/HWDGE: all DMA.
"""
import sys
import math

sys.path.insert(0, '/opt/trn_rl_repo')

import numpy as np
import concourse.bass as bass
import concourse.mybir as mybir
from concourse.bass_utils import run_bass_kernel_spmd

FP32 = mybir.dt.float32
FP32R = mybir.dt.float32r
EXP = mybir.ActivationFunctionType.Exp
MULT = mybir.AluOpType.mult

B, H, S, D = 4, 16, 2048, 64
N_CORES = 8
NH = (B * H) // N_CORES     # heads per core
QT_W = 512                  # supertile width (queries)
NP = 6                      # p-tile ring slots


def _copy_engine(c):
    """Engine for the psum->sbuf copy of transposed-p chunk c."""
    return "D" if c % 3 == 2 else "A"


def _round_fp32r(x):
    u = np.ascontiguousarray(x).view(np.uint32)
    lsb = (u >> 12) & np.uint32(1)
    u2 = (u + np.uint32(0x7FF) + lsb) & np.uint32(0xFFFFF000)
    return u2.view(np.float32)


def _build_nc(nh, s, n_cores):
    nj = QT_W // 128            # q-tiles per supertile (4)
    nch = s // 128              # k-chunks
    nst = s // QT_W             # supertiles
    niter = nst * nh
    nsec = nch // nj            # chunks per interleave section

    copy_eng = [_copy_engine(c) for c in range(nch)]
    ncopA = copy_eng.count("A")
    ncopD = copy_eng.count("D")
    cumA = np.cumsum([1 if e == "A" else 0 for e in copy_eng])
    cumD = np.cumsum([1 if e == "D" else 0 for e in copy_eng])

    def cnt_after(e):
        """(engine, per-engine count) after global copy event e."""
        if e < 0:
            return None
        it, c = divmod(e, nch)
        if copy_eng[c] == "A":
            return "A", ncopA * it + int(cumA[c])
        return "D", ncopD * it + int(cumD[c])

    nc = bass.Bass("TRN2", target_bir_lowering=False, debug=False,
                   num_devices=n_cores)

    qt = nc.dram_tensor("qt", [nh, D, s], FP32R, kind="ExternalInput").ap()
    kt = nc.dram_tensor("kt", [D, nh * s], FP32R, kind="ExternalInput").ap()
    v = nc.dram_tensor("v", [128, nh * (s // 128) * D], FP32R,
                       kind="ExternalInput").ap()
    sg = nc.dram_tensor("sg", [s, s], FP32, kind="ExternalInput").ap()
    p_out = nc.dram_tensor("p_out", [nh, s, s], FP32R,
                           kind="ExternalOutput").ap()
    o_out = nc.dram_tensor("o_out", [nh, D, s], FP32,
                           kind="ExternalOutput").ap()

    kt_all = nc.alloc_sbuf_tensor("kt_all", [D, nh * s], FP32R).ap()
    v_all = nc.alloc_sbuf_tensor("v_all", [128, nh * (s // 128) * D],
                                 FP32R).ap()
    sg_sb = nc.alloc_sbuf_tensor("sg_sb", [128, nj * s], FP32).ap()
    qs = [nc.alloc_sbuf_tensor(f"qs{x}", [D, QT_W], FP32R).ap()
          for x in range(2)]
    u = [nc.alloc_sbuf_tensor(f"u{x}", [128, s], FP32).ap() for x in range(2)]
    sm = [nc.alloc_sbuf_tensor(f"sm{x}", [128, 1], FP32).ap()
          for x in range(2)]
    iv = [nc.alloc_sbuf_tensor(f"iv{x}", [128, 1], FP32).ap()
          for x in range(2)]
    p = [nc.alloc_sbuf_tensor(f"p{x}", [128, s], FP32R).ap()
         for x in range(NP)]
    uT = [nc.alloc_sbuf_tensor(f"uT{x}", [128, QT_W], FP32R).ap()
          for x in range(2)]
    oT = [nc.alloc_sbuf_tensor(f"oT{x}", [D, QT_W], FP32).ap()
          for x in range(2)]
    identf = nc.alloc_sbuf_tensor("identf", [128, 128], FP32).ap()
    ident = nc.alloc_sbuf_tensor("ident", [128, 128], FP32R).ap()

    ps_s = nc.alloc_psum_tensor("ps_s", [128, s], FP32).ap()
    ps_t = [nc.alloc_psum_tensor(f"ps_t{x}", [128, QT_W], FP32R).ap()
            for x in range(2)]
    ps_o = nc.alloc_psum_tensor("ps_o", [D, QT_W], FP32).ap()

    def head_of(i):
        return i % nh

    def js_of(i):
        return i // nh

    from contextlib import ExitStack
    with ExitStack() as _ctx:
        block = _ctx.enter_context(nc.Block())
        _sem_names = ["dInit", "dQ0", "dQ1", "dS", "dP0", "dP1", "dP2",
                      "dP3", "dP4", "dP5", "dO0", "dO1", "sMM", "sTR",
                      "sAV", "sEXP", "sSTT", "sCPA", "sCPD", "sOC",
                      "sGP", "sINI"]
        _s = {n: _ctx.enter_context(nc.semaphore(n)) for n in _sem_names}
        (dInit, dQ0, dQ1, dS, dP0, dP1, dP2, dP3, dP4, dP5, dO0, dO1,
         sMM, sTR, sAV, sEXP, sSTT, sCPA, sCPD, sOC, sGP, sINI) = (
            _s[n] for n in _sem_names)

        sems = {"A": sCPA, "D": sCPD}
        dQs = [dQ0, dQ1]
        dPs = [dP0, dP1, dP2, dP3, dP4, dP5]
        dOs = [dO0, dO1]

        # ---------------- SP / DMA ----------------
        @block.sync
        def _(sync):
            sync.dma_start(kt_all[:], kt[:]).then_inc(dInit, 16)
            sync.dma_start(v_all[:], v[:]).then_inc(dInit, 16)
            sync.dma_start(
                sg_sb[:].rearrange("p (j k) -> p j k", j=nj),
                sg[0:QT_W, :].rearrange("(j p) k -> p j k", p=128),
            ).then_inc(dS, 16)
            sync.dma_start(qs[0][:], qt[0, :, 0:QT_W]).then_inc(dQ0, 16)
            for i in range(niter):
                js, h = js_of(i), head_of(i)
                if i + 1 < niter:
                    js1, h1 = js_of(i + 1), head_of(i + 1)
                    if i >= 1:
                        sync.wait_ge(sMM, nj * i)
                    sync.dma_start(
                        qs[(i + 1) % 2][:],
                        qt[h1, :, js1 * QT_W:(js1 + 1) * QT_W],
                    ).then_inc(dQs[(i + 1) % 2], 16)
                    if h1 == 0:
                        sync.wait_ge(sSTT, nj * (i + 1))
                        sync.dma_start(
                            sg_sb[:].rearrange("p (j k) -> p j k", j=nj),
                            sg[js1 * QT_W:(js1 + 1) * QT_W, :]
                            .rearrange("(j p) k -> p j k", p=128),
                        ).then_inc(dS, 16)
                for j in range(nj):
                    g = nj * i + j
                    sync.wait_ge(sSTT, g + 1)
                    q0 = js * QT_W + j * 128
                    sync.dma_start(p_out[h, q0:q0 + 128, :],
                                   p[g % NP][:]).then_inc(dPs[g % NP], 16)
                if i >= 1:
                    sync.wait_ge(sOC, i)
                    jsp, hp = js_of(i - 1), head_of(i - 1)
                    sync.dma_start(
                        o_out[hp, :, jsp * QT_W:(jsp + 1) * QT_W],
                        oT[(i - 1) % 2][:],
                    ).then_inc(dOs[(i - 1) % 2], 16)
            sync.wait_ge(sOC, niter)
            jsp, hp = js_of(niter - 1), head_of(niter - 1)
            sync.dma_start(
                o_out[hp, :, jsp * QT_W:(jsp + 1) * QT_W],
                oT[(niter - 1) % 2][:],
            ).then_inc(dOs[(niter - 1) % 2], 16)

        # ---------------- GPSIMD: identity init ----------------
        @block.gpsimd
        def _(g):
            g.memset(identf[:], 0.0)
            g.affine_select(
                out=identf[:], in_=identf[:],
                compare_op=mybir.AluOpType.not_equal,
                fill=1.0, base=0, pattern=[[-1, 128]], channel_multiplier=1,
            ).then_inc(sGP, 1)

        # ---------------- PE ----------------
        def emit_chunk_T(t, i_prev, c):
            """Transposes of iter i_prev chunk c (4 blocks)."""
            e = nch * i_prev + c
            war = cnt_after(e - 2)
            if war is not None:
                t.wait_ge(sems[war[0]], war[1])
            for jj in range(nj):
                inst = nc.tensor.transpose(
                    ps_t[c % 2][:, jj * 128:(jj + 1) * 128],
                    p[(nj * i_prev + jj) % NP][:, c * 128:(c + 1) * 128],
                    ident[:],
                )
                if jj == nj - 1:
                    inst.then_inc(sTR, 1)

        def emit_av(t, i_prev, c):
            """AV matmul consuming copy of chunk c of iter i_prev."""
            h = head_of(i_prev)
            cp = cnt_after(nch * i_prev + c)
            t.wait_ge(sems[cp[0]], cp[1])
            nc.tensor.matmul(
                ps_o[:],
                v_all[:, (h * nch + c) * D:(h * nch + c + 1) * D],
                uT[c % 2][:],
                start=(c == 0), stop=(c == nch - 1),
            ).then_inc(sAV, 1)

        @block.tensor
        def _(t):
            t.wait_ge(dInit, 32)
            t.wait_ge(sINI, 1)
            for i in range(niter + 1):
                run_mm = i < niter
                run_tav = i >= 1
                if run_mm:
                    t.wait_ge(dQs[i % 2], 16 * (i // 2 + 1))
                h = head_of(i) if run_mm else 0
                for j in range(nj):
                    if run_mm:
                        g = nj * i + j
                        if g >= 1:
                            t.wait_ge(sEXP, g)   # ps_s WAR
                        for n in range(s // 512):
                            inst = nc.tensor.matmul(
                                ps_s[:, n * 512:(n + 1) * 512],
                                qs[i % 2][:, j * 128:(j + 1) * 128],
                                kt_all[:, h * s + n * 512:
                                       h * s + (n + 1) * 512],
                                start=True, stop=True,
                            )
                            if n == s // 512 - 1:
                                inst.then_inc(sMM, 1)
                    if run_tav and j == 0:
                        t.wait_ge(sSTT, nj * i)  # p of iter i-1 written
                        if i >= 2:
                            t.wait_ge(sOC, i - 1)  # ps_o WAR vs oT copy(i-2)
                    if run_tav:
                        for c in range(j * nsec, (j + 1) * nsec):
                            emit_chunk_T(t, i - 1, c)
                            if c >= 1:
                                emit_av(t, i - 1, c - 1)
                if run_tav:
                    emit_av(t, i - 1, nch - 1)

        # ---------------- ACT ----------------
        @block.scalar
        def _(a):
            for i in range(niter + 1):
                run_mm = i < niter
                run_tav = i >= 1
                for j in range(nj):
                    if run_mm:
                        g = nj * i + j
                        a.wait_ge(sMM, g + 1)
                        if g - 1 >= 1:
                            a.wait_ge(sSTT, g - 1)   # u/sm slot WAR
                        nc.scalar.activation(
                            u[j % 2][:], ps_s[:], EXP,
                            scale=1.0 / math.sqrt(D),
                            accum_out=sm[j % 2][:],
                        ).then_inc(sEXP, 1)
                    if run_tav:
                        for c in range(j * nsec, (j + 1) * nsec):
                            if copy_eng[c] != "A":
                                continue
                            e = nch * (i - 1) + c
                            a.wait_ge(sTR, e + 1)
                            if e - 1 >= 1:
                                a.wait_ge(sAV, e - 1)  # uT WAR vs AV(c-2)
                            nc.scalar.copy(uT[c % 2][:],
                                           ps_t[c % 2][:]).then_inc(sCPA, 1)

        # ---------------- DVE ----------------
        @block.vector
        def _(d):
            d.wait_ge(sGP, 1)
            nc.vector.tensor_copy(ident[:], identf[:]).then_inc(sINI, 1)
            for i in range(niter + 1):
                run_mm = i < niter
                run_tav = i >= 1
                js = js_of(i) if run_mm else 0
                h = head_of(i) if run_mm else 0
                for j in range(nj):
                    if run_tav:
                        for c in range(j * nsec, (j + 1) * nsec):
                            if copy_eng[c] != "D":
                                continue
                            e = nch * (i - 1) + c
                            d.wait_ge(sTR, e + 1)
                            if e - 1 >= 1:
                                d.wait_ge(sAV, e - 1)
                            nc.vector.tensor_copy(
                                uT[c % 2][:],
                                ps_t[c % 2][:]).then_inc(sCPD, 1)
                    if run_mm:
                        g = nj * i + j
                        d.wait_ge(sEXP, g + 1)
                        if h == 0 and j == 0:
                            d.wait_ge(dS, 16 * (js + 1))
                        gp = g - NP                  # p slot predecessor
                        if gp >= 0:
                            d.wait_ge(dPs[g % NP], 16 * (gp // NP + 1))
                            d.wait_ge(sTR, nch * (gp // nj + 1))
                        nc.vector.reciprocal(iv[j % 2][:], sm[j % 2][:])
                        d.drain()
                        nc.vector.scalar_tensor_tensor(
                            p[g % NP][:], u[j % 2][:], iv[j % 2][:],
                            sg_sb[:, j * s:(j + 1) * s],
                            MULT, MULT,
                        ).then_inc(sSTT, 1)
                if run_tav:
                    d.wait_ge(sAV, nch * i)
                    if i >= 3:
                        d.wait_ge(dOs[(i - 1) % 2],
                                  16 * ((i - 3) // 2 + 1))  # oT slot WAR
                    nc.vector.tensor_copy(
                        oT[(i - 1) % 2][:], ps_o[:]).then_inc(sOC, 1)

    return nc


_NC_CACHE = {}


def _get_nc(nh, s, n_cores=N_CORES):
    key = (nh, s, n_cores)
    if key not in _NC_CACHE:
        _NC_CACHE[key] = _build_nc(nh, s, n_cores)
    return _NC_CACHE[key]


def _prep_core_inputs(query, key, value, time, mask_logits, n_cores=N_CORES):
    b, h, s, d = query.shape
    nh = (b * h) // n_cores
    kt_full = key + time
    q_f = query.reshape(b * h, s, d)
    kt_f = kt_full.reshape(b * h, s, d)
    v_f = value.reshape(b * h, s, d)
    sig = 1.0 / (1.0 + np.exp(-mask_logits.astype(np.float64)))
    sig = sig.astype(np.float32)

    in_maps = []
    for c in range(n_cores):
        sl = slice(c * nh, (c + 1) * nh)
        qt = _round_fp32r(np.ascontiguousarray(q_f[sl].transpose(0, 2, 1)))
        # kt layout: [D, nh*S]: kt[d, h*S + s] = (K+T)[h, s, d]
        ktp = _round_fp32r(np.ascontiguousarray(
            kt_f[sl].transpose(2, 0, 1).reshape(d, nh * s)))
        vv = v_f[sl].reshape(nh, s // 128, 128, d)
        vp = _round_fp32r(np.ascontiguousarray(
            vv.transpose(2, 0, 1, 3).reshape(128, nh * (s // 128) * d)))
        in_maps.append({"qt": qt, "kt": ktp, "v": vp, "sg": sig})
    return in_maps


def _reference_fallback(query, key, value, time, attn_mask, mask_logits):
    d = query.shape[-1]
    inv = 1.0 / math.sqrt(d)
    scores = np.einsum("bhqd,bhkd->bhqk", query, key) * inv
    scores = np.where(attn_mask, scores, np.float32(-1e9))
    scores = scores + np.einsum("bhqd,bhkd->bhqk", query, time) * inv
    scores = scores - scores.max(axis=-1, keepdims=True)
    e = np.exp(scores)
    p = e / e.sum(axis=-1, keepdims=True)
    p = (p * (1.0 / (1.0 + np.exp(-mask_logits)))).astype(np.float32)
    out = np.einsum("bhqk,bhkd->bhqd", p, value).astype(np.float32)
    return out, p


def kernel(query, key, value, time, attn_mask, mask_logits, _trace=False,
           _n_cores=N_CORES):
    query = np.asarray(query, dtype=np.float32)
    key = np.asarray(key, dtype=np.float32)
    value = np.asarray(value, dtype=np.float32)
    time = np.asarray(time, dtype=np.float32)
    attn_mask = np.asarray(attn_mask)
    mask_logits = np.asarray(mask_logits, dtype=np.float32)

    if not bool(attn_mask.all()):
        return _reference_fallback(query, key, value, time, attn_mask,
                                   mask_logits)

    b, h, s, d = query.shape
    nh = (b * h) // _n_cores
    nc = _get_nc(nh, s, _n_cores)
    in_maps = _prep_core_inputs(query, key, value, time, mask_logits,
                                _n_cores)
    res = run_bass_kernel_spmd(nc, in_maps, list(range(_n_cores)),
                               trace=_trace)

    p_attn = np.empty((b * h, s, s), dtype=np.float32)
    out = np.empty((b * h, s, d), dtype=np.float32)
    for c in range(_n_cores):
        r = res.results[c]
        p_attn[c * nh:(c + 1) * nh] = r["p_out"]
        out[c * nh:(c + 1) * nh] = r["o_out"].transpose(0, 2, 1)
    p_attn = p_attn.reshape(b, h, s, s)
    out = out.reshape(b, h, s, d)
    if _trace:
        kernel._last_results = res
    return out, p_attn
